# revision 9
# baseline (speedup 1.0000x reference)
"""CrossWinAttention Trainium2 kernel, v2 (bf16 DMA + [q,dh] attention layout).

Computes, for each of 256 independent (x,y) windows:
  LN -> Q/K/V projections -> 4-head attention over T=384 tokens
  -> output projection -> mean over the N=6 slices.

Sharding: 8 cores x 32 windows. LN affine and linear biases folded into
weights host-side; 1/N mean folded into Wp. Inputs are DMA'd as bf16
(halves HBM traffic; comfortably inside the rel-err budget).

Cost-model-driven engine assignment (PE matmul cost = output free size;
GPSIMD cannot touch PSUM; DMA cannot read PSUM):
  - attention output A in [query, head-dim] layout: out [128q,32] costs
    32 rows/instruction instead of 384 -> A path ~4x cheaper; softmax
    denominators are 1-column matmuls; 1/den is a per-partition scalar.
  - x^T and A^T via one dma_start_transpose each (SBUF->SBUF xbar,
    14ns/tile) instead of PE transposes + PSUM->SBUF bounce copies.
  - LN apply on GPSIMD (SBUF->SBUF), stats on DVE, exp on ACT with an
    optional Schraudolph tail on DVE (exp(x) ~ bitcast(int16(x*a+b))).
  - output stays z^T [dim, wq]; the host transposes on unshard.
"""

import ml_dtypes
import numpy as np

import concourse.bass as bass
import concourse.tile as tile
from concourse import mybir
from concourse.bass_utils import run_bass_kernel_spmd

# Problem shape (hardcoded per spec)
B, N, X, Y, W1, W2 = 1, 6, 16, 16, 8, 8
DIM, HEADS, DH = 128, 4, 32
INNER = HEADS * DH
T = N * W1 * W2          # 384 tokens per window
WQ = W1 * W2             # 64 tokens per n-slice
NCORES = 8
WIN_PER_CORE = (X // NCORES) * Y   # 32
EPS = 1e-5
SCALE = DH ** -0.5
F32 = mybir.dt.float32
BF16 = mybir.dt.bfloat16
I16 = mybir.dt.int16
NPBF = ml_dtypes.bfloat16
ACT = mybir.ActivationFunctionType
ALU = mybir.AluOpType

TCH = T // 128           # 3 token chunks of 128

# Schraudolph exp in bf16: exp(x) ~ bitcast_bf16(int16(x*EXPA + EXPB))
EXPA = 128.0 / np.log(2.0)
EXPB = 16249.5
# columns of the last key-chunk's scores that take the DVE exp path
DVE_EXP_COLS = 64

LAST_RESULT = None       # BassKernelResults of the most recent kernel() call


def host_consts():
    identbf = np.eye(128, dtype=np.float32).astype(NPBF)
    ones_col = np.ones((128, 1), np.float32).astype(NPBF)
    return identbf, ones_col


def build(n_win: int, with_bias: bool):
    """Build the per-core Bass program. Inputs are per-core shards.

    xin: [n_win, 128, 3, TCH, DIM] bf16 (q,k,v tensor-major, token chunks)
    out: [n_win, DIM, WQ] fp32  (z^T per window; host transposes)
    """
    nc = bass.Bass()

    xin_d = nc.dram_tensor("xin", [n_win, 128, 3, TCH, DIM], BF16,
                           kind="ExternalInput")
    wq_d = nc.dram_tensor("wq", [DIM, INNER], BF16, kind="ExternalInput")
    wk_d = nc.dram_tensor("wk", [DIM, INNER], BF16, kind="ExternalInput")
    wv_d = nc.dram_tensor("wv", [DIM, INNER], BF16, kind="ExternalInput")
    wp_d = nc.dram_tensor("wp", [INNER, DIM], BF16, kind="ExternalInput")
    ones_d = nc.dram_tensor("ones_col", [128, 1], BF16, kind="ExternalInput")
    if with_bias:
        bq_d = nc.dram_tensor("bq", [INNER, 1], F32, kind="ExternalInput")
        bk_d = nc.dram_tensor("bk", [INNER, 1], F32, kind="ExternalInput")
        bv_d = nc.dram_tensor("bv_row", [1, INNER], BF16, kind="ExternalInput")
        bp_d = nc.dram_tensor("bp6", [DIM, 1], F32, kind="ExternalInput")
    out_d = nc.dram_tensor("out", [n_win, DIM, WQ], F32, kind="ExternalOutput")

    with tile.TileContext(nc) as tc:
        with (
            tc.tile_pool(name="singles", bufs=1) as singles,
            tc.tile_pool(name="xbuf", bufs=2) as xbuf,
            tc.tile_pool(name="lnb", bufs=2) as lnb,
            tc.tile_pool(name="stats", bufs=2) as statp,
            tc.tile_pool(name="xt", bufs=2) as xtp,
            tc.tile_pool(name="qkv", bufs=2) as qkvp,
            tc.tile_pool(name="esb", bufs=2) as esbp,
            tc.tile_pool(name="small", bufs=2) as smallp,
            tc.tile_pool(name="ps_p", bufs=2, space="PSUM") as ps_p,
            tc.tile_pool(name="ps_s", bufs=1, space="PSUM") as ps_s,
            tc.tile_pool(name="ps_a", bufs=1, space="PSUM") as ps_a,
        ):
            # ---- constants / weights ----
            ones_col = singles.tile([128, 1], BF16)
            nc.sync.dma_start(ones_col, ones_d[:, :])
            eps_sb = singles.tile([128, 1], F32)
            nc.vector.memset(eps_sb, EPS)
            zero_sb = singles.tile([128, 1], F32)
            nc.vector.memset(zero_sb, 0.0)
            w_sb = {}
            for nm, d in (("q", wq_d), ("k", wk_d), ("v", wv_d), ("p", wp_d)):
                w_sb[nm] = singles.tile([128, 128], BF16, name=f"w{nm}", tag=f"w{nm}")
                nc.sync.dma_start(w_sb[nm], d[:, :])
            if with_bias:
                bq_sb = singles.tile([INNER, 1], F32)
                nc.sync.dma_start(bq_sb, bq_d[:, :])
                bk_sb = singles.tile([INNER, 1], F32)
                nc.sync.dma_start(bk_sb, bk_d[:, :])
                bv_sb = singles.tile([1, INNER], BF16)
                nc.sync.dma_start(bv_sb, bv_d[:, :])
                bp_sb = singles.tile([DIM, 1], F32)
                nc.sync.dma_start(bp_sb, bp_d[:, :])
                ones_row128 = singles.tile([1, 128], BF16)
                nc.vector.memset(ones_row128, 1.0)

            # software-pipelined input loads: issue window w+1's DMA during
            # window w so it never queues behind w's late-stage SP DMAs
            x_tiles = {}
            x_tiles[0] = xbuf.tile([128, 3, TCH, DIM], BF16, tag="x", name="x")
            nc.sync.dma_start(x_tiles[0][:, :, :, :], xin_d[0, :, :, :, :])
            for w in range(n_win):
                x_sb = x_tiles.pop(w)
                if w + 1 < n_win:
                    x_tiles[w + 1] = xbuf.tile([128, 3, TCH, DIM], BF16,
                                               tag="x", name="x")
                    nc.sync.dma_start(x_tiles[w + 1][:, :, :, :],
                                      xin_d[w + 1, :, :, :, :])
                stats6 = statp.tile([128, 9, 6], F32, tag="st6")
                for ti in range(3):
                    for c in range(TCH):
                        nc.vector.bn_stats(
                            out=stats6[:, 3 * ti + c, :], in_=x_sb[:, ti, c, :]
                        )
                mv = statp.tile([128, 9, 2], F32, tag="mv")
                for g in range(9):
                    nc.vector.bn_aggr(out=mv[:, g, :], in_=stats6[:, g, :])
                # rstd = exp(-0.5 * ln(var + EPS))
                lnv = statp.tile([128, 9], F32, tag="lnv")
                nc.scalar.activation(
                    out=lnv, in_=mv[:, :, 1], func=ACT.Ln, bias=eps_sb, scale=1.0
                )
                rstd = statp.tile([128, 9], F32, tag="rstd")
                nc.scalar.activation(
                    out=rstd, in_=lnv, func=ACT.Exp, bias=zero_sb, scale=-0.5
                )
                # LN apply on GPSIMD (SBUF->SBUF)
                xl = lnb.tile([128, 3, TCH, DIM], BF16, tag="xl", name="xl")
                for ti in range(3):
                    for c in range(TCH):
                        g = 3 * ti + c
                        nc.gpsimd.tensor_scalar(
                            out=xl[:, ti, c, :], in0=x_sb[:, ti, c, :],
                            scalar1=mv[:, g, 0:1], scalar2=rstd[:, g:g + 1],
                            op0=ALU.subtract, op1=ALU.mult,
                        )

                # ---- x^T via one SBUF->SBUF xbar DMA transpose ----
                # xt[:, ti, c, :] = xl[:, ti, c, :]^T  -> [128(d), 3, TCH, 128(t)]
                xt_ = xtp.tile([128, 3, TCH, 128], BF16, tag="xt", name="xt")
                nc.sync.dma_start_transpose(
                    xt_[:, :, :, :], xl[:, :, :, :]
                )

                # ---- projections (bf16 weights, fp32 PSUM) ----
                qT = qkvp.tile([128, T], BF16, tag="qT")
                kT = qkvp.tile([128, T], BF16, tag="kT")
                for ti, (nm, dst) in enumerate((("q", qT), ("k", kT))):
                    pps = ps_p.tile([128, T], F32, tag="pp")
                    nc.tensor.matmul(pps, w_sb[nm], xt_[:, ti, :, :])
                    if with_bias:
                        bb = bq_sb if nm == "q" else bk_sb
                        nc.vector.tensor_scalar(
                            out=dst, in0=pps, scalar1=bb, scalar2=None, op0=ALU.add
                        )
                    else:
                        nc.vector.tensor_copy(dst, pps)
                v_sb = qkvp.tile([128, TCH, DIM], BF16, tag="v")
                vps = ps_p.tile([128, TCH, DIM], F32, tag="pp")
                for c in range(TCH):
                    nc.tensor.matmul(
                        vps[:, c, :], xt_[:, 2, c, :], w_sb["v"]
                    )
                    if with_bias:
                        nc.tensor.matmul(
                            vps[:, c, :], ones_row128, bv_sb, start=False
                        )
                nc.vector.tensor_copy(v_sb, vps)

                # ---- attention: scores -> exp -> A[q, dh] + den ----
                a_ps = ps_a.tile([128, TCH, HEADS, DH], F32, tag="A")
                dz_ps = ps_a.tile([128, 12 + WQ], F32, tag="dz")
                # head-pair tiles double-buffer so exp(pair p) overlaps
                # scores(pair p+1) on the PE
                for ck in range(TCH):
                    for pr in range(2):
                        s_ps = ps_s.tile([128, 2, 512], F32, tag="S", bufs=2)
                        for hh in range(2):
                            h = 2 * pr + hh
                            nc.tensor.matmul(
                                s_ps[:, hh, 0:T],
                                kT[32 * h:32 * h + 32, 128 * ck:128 * (ck + 1)],
                                qT[32 * h:32 * h + 32, :],
                                tile_position=(32 * h, 0),
                            )
                        e_sb = esbp.tile([128, 2, T], BF16, tag=f"E{ck}{pr}")
                        ncols = DVE_EXP_COLS if ck == TCH - 1 else 0
                        if ncols:
                            e_i = e_sb.bitcast(I16)
                            nc.vector.tensor_scalar(
                                out=e_i[:, :, T - ncols:T],
                                in0=s_ps[:, :, T - ncols:T],
                                scalar1=EXPA * SCALE, scalar2=EXPB,
                                op0=ALU.mult, op1=ALU.add,
                            )
                        nc.scalar.activation(
                            out=e_sb[:, :, 0:T - ncols],
                            in_=s_ps[:, :, 0:T - ncols],
                            func=ACT.Exp, bias=zero_sb, scale=SCALE,
                        )
                        for cq in range(TCH):
                            for hh in range(2):
                                h = 2 * pr + hh
                                # PSUM start=True zeroes the whole 2KB bank
                                # (ZERO_REGION_SIZE): issue it exactly once
                                # per bank per window, others accumulate.
                                first = (ck == 0 and pr == 0 and cq == 0
                                         and hh == 0)
                                sp = (ck == TCH - 1)
                                nc.tensor.matmul(
                                    a_ps[:, cq, h, :],
                                    e_sb[:, hh, 128 * cq:128 * (cq + 1)],
                                    v_sb[:, ck, 32 * h:32 * h + 32],
                                    start=first, stop=sp, skip_group_check=True,
                                )
                                nc.tensor.matmul(
                                    dz_ps[:, 4 * cq + h:4 * cq + h + 1],
                                    e_sb[:, hh, 128 * cq:128 * (cq + 1)],
                                    ones_col,
                                    start=first, stop=sp, skip_group_check=True,
                                )

                # ---- normalize (per-partition scalars, bcast over dh) ----
                r_sb = smallp.tile([128, 12], F32, tag="r")
                nc.vector.reciprocal(r_sb, dz_ps[:, 0:12])
                a_sb = smallp.tile([128, TCH, HEADS, DH], BF16, tag="asb")
                for cq in range(TCH):
                    with nc.allow_low_precision(reason="attn out to bf16"):
                        nc.vector.tensor_tensor(
                            out=a_sb[:, cq, :, :],
                            in0=a_ps[:, cq, :, :],
                            in1=r_sb[:, 4 * cq:4 * cq + 4, None]
                                .broadcast_to([128, HEADS, DH]),
                            op=ALU.mult,
                        )
                # A^T via xbar DMA transpose: at[:, cq, :] = a_sb[:, cq, :]^T
                at_sb = smallp.tile([128, TCH, 128], BF16, tag="atsb")
                nc.sync.dma_start_transpose(
                    at_sb[:, :, :], a_sb[:, :, :, :]
                )

                # ---- output projection with folded n-mean, store z^T ----
                zt_view = dz_ps[:, 12:12 + WQ]
                for n in range(N):
                    cq, t0 = divmod(n * WQ, 128)
                    # dz bank was zeroed by the first den matmul's start
                    nc.tensor.matmul(
                        zt_view, w_sb["p"], at_sb[:, cq, t0:t0 + WQ],
                        start=False, stop=(n == N - 1), skip_group_check=True,
                    )
                zfin = smallp.tile([128, WQ], F32, tag="zfin")
                if with_bias:
                    nc.vector.tensor_scalar(
                        out=zfin, in0=zt_view, scalar1=bp_sb, scalar2=None,
                        op0=ALU.add,
                    )
                else:
                    nc.vector.tensor_copy(zfin, zt_view)
                nc.gpsimd.dma_start(out_d[w, :, :], zfin)

    return nc


def split_multi_waits(nc):
    """Walrus encodes at most one sem-wait per instruction on this toolchain;
    move extra waits onto same-engine NoOp carriers placed just before."""
    k = 0
    for f in nc.m.functions:
        for blk in f.blocks:
            new = []
            for inst in blk.instructions:
                si = getattr(inst, "sync_info", None)
                if si and si.on_wait and len(si.on_wait) > 1:
                    waits = list(si.on_wait)
                    for w in waits[:-1]:
                        nop = mybir.InstNoOp(
                            name=f"{inst.name}_wsplit{k}", ins=[], outs=[]
                        )
                        k += 1
                        nop.engine = inst.engine
                        nop.sync_info = mybir.SyncInfo(on_wait=[w], on_update=[])
                        new.append(nop)
                    si.on_wait = [waits[-1]]
                new.append(inst)
            blk.instructions[:] = new
    return nc


def fold_params(inp):
    folded = {}
    for nm in ("q", "k", "v"):
        g = inp[f"ln_{nm}_g"]
        bb = inp[f"ln_{nm}_b"]
        W = inp[f"W{nm}"]
        folded[f"W{nm}"] = np.ascontiguousarray(g[:, None] * W)
        folded[f"b{nm}"] = inp[f"b{nm}"] + bb @ W
    folded["Wp6"] = np.ascontiguousarray(inp["Wp"] / N)
    folded["bp6"] = inp["bp"] / N
    return folded


def make_base_inputs(folded, with_bias):
    identbf, ones_col = host_consts()
    base = {
        "wq": folded["Wq"].astype(NPBF), "wk": folded["Wk"].astype(NPBF),
        "wv": folded["Wv"].astype(NPBF), "wp": folded["Wp6"].astype(NPBF),
        "ones_col": ones_col,
    }
    if with_bias:
        base["bq"] = folded["bq"].reshape(INNER, 1).astype(np.float32)
        base["bk"] = folded["bk"].reshape(INNER, 1).astype(np.float32)
        base["bv_row"] = folded["bv"].reshape(1, INNER).astype(NPBF)
        base["bp6"] = folded["bp6"].reshape(DIM, 1).astype(np.float32)
    return base


def make_xin(inp, core):
    """Pack core `core`'s windows: [n_win, 128, 3, TCH, DIM] bf16."""
    xrows = X // NCORES
    parts = []
    for key in ("q", "k", "v"):
        sh = inp[key][0, :, xrows * core:xrows * (core + 1)]  # [N,2,Y,W1,W2,D]
        sh = sh.reshape(N, WIN_PER_CORE, WQ, DIM).transpose(1, 0, 2, 3)
        sh = sh.reshape(WIN_PER_CORE, TCH, 128, DIM).transpose(0, 2, 1, 3)
        parts.append(sh)  # [n_win, 128, TCH, DIM]
    xin = np.stack(parts, axis=2)  # [n_win, 128, 3, TCH, DIM]
    return np.ascontiguousarray(xin.astype(NPBF))


def kernel(**inputs) -> np.ndarray:
    inp = {k: np.ascontiguousarray(np.asarray(v, dtype=np.float32))
           for k, v in inputs.items()}

    folded = fold_params(inp)
    with_bias = any(
        np.abs(folded[b]).max() > 0 for b in ("bq", "bk", "bv", "bp6")
    )

    nc = build(WIN_PER_CORE, with_bias)
    nc.finalize()
    split_multi_waits(nc)

    base = make_base_inputs(folded, with_bias)
    in_maps = []
    for c in range(NCORES):
        m = dict(base)
        m["xin"] = make_xin(inp, c)
        in_maps.append(m)

    res = run_bass_kernel_spmd(nc, in_maps, core_ids=list(range(NCORES)))
    global LAST_RESULT
    LAST_RESULT = res
    outs = res.results
    xrows = X // NCORES
    full = np.zeros((B, X, Y, W1, W2, DIM), np.float32)
    for c in range(NCORES):
        o = np.asarray(outs[c]["out"])          # [n_win, DIM, WQ]
        o = o.transpose(0, 2, 1)                # [n_win, WQ, DIM]
        o = o.reshape(xrows, Y, W1, W2, DIM)
        full[0, xrows * c:xrows * (c + 1)] = o
    return full


# revision 11
# speedup vs baseline: 1.0020x; 1.0020x over previous
"""CrossWinAttention Trainium2 kernel, v2 (bf16 DMA + [q,dh] attention layout).

Computes, for each of 256 independent (x,y) windows:
  LN -> Q/K/V projections -> 4-head attention over T=384 tokens
  -> output projection -> mean over the N=6 slices.

Sharding: 8 cores x 32 windows. LN affine and linear biases folded into
weights host-side; 1/N mean folded into Wp. Inputs are DMA'd as bf16
(halves HBM traffic; comfortably inside the rel-err budget).

Cost-model-driven engine assignment (PE matmul cost = output free size;
GPSIMD cannot touch PSUM; DMA cannot read PSUM):
  - attention output A in [query, head-dim] layout: out [128q,32] costs
    32 rows/instruction instead of 384 -> A path ~4x cheaper; softmax
    denominators are 1-column matmuls; 1/den is a per-partition scalar.
  - x^T and A^T via one dma_start_transpose each (SBUF->SBUF xbar,
    14ns/tile) instead of PE transposes + PSUM->SBUF bounce copies.
  - LN apply on GPSIMD (SBUF->SBUF), stats on DVE, exp on ACT with an
    optional Schraudolph tail on DVE (exp(x) ~ bitcast(int16(x*a+b))).
  - output stays z^T [dim, wq]; the host transposes on unshard.
"""

import ml_dtypes
import numpy as np

import concourse.bass as bass
import concourse.tile as tile
from concourse import mybir
from concourse.bass_utils import run_bass_kernel_spmd

# Problem shape (hardcoded per spec)
B, N, X, Y, W1, W2 = 1, 6, 16, 16, 8, 8
DIM, HEADS, DH = 128, 4, 32
INNER = HEADS * DH
T = N * W1 * W2          # 384 tokens per window
WQ = W1 * W2             # 64 tokens per n-slice
NCORES = 8
WIN_PER_CORE = (X // NCORES) * Y   # 32
EPS = 1e-5
SCALE = DH ** -0.5
F32 = mybir.dt.float32
BF16 = mybir.dt.bfloat16
I16 = mybir.dt.int16
NPBF = ml_dtypes.bfloat16
ACT = mybir.ActivationFunctionType
ALU = mybir.AluOpType

TCH = T // 128           # 3 token chunks of 128

# Schraudolph exp in bf16: exp(x) ~ bitcast_bf16(int16(x*EXPA + EXPB))
EXPA = 128.0 / np.log(2.0)
EXPB = 16249.5
# columns of the last key-chunk's scores that take the DVE exp path
DVE_EXP_COLS = 64

LAST_RESULT = None       # BassKernelResults of the most recent kernel() call


def host_consts():
    identbf = np.eye(128, dtype=np.float32).astype(NPBF)
    ones_col = np.ones((128, 1), np.float32).astype(NPBF)
    return identbf, ones_col


def build(n_win: int, with_bias: bool):
    """Build the per-core Bass program. Inputs are per-core shards.

    xin: [n_win, 128, 3, TCH, DIM] bf16 (q,k,v tensor-major, token chunks)
    out: [n_win, DIM, WQ] fp32  (z^T per window; host transposes)
    """
    nc = bass.Bass()

    xin_d = nc.dram_tensor("xin", [n_win, 128, 3, TCH, DIM], BF16,
                           kind="ExternalInput")
    wq_d = nc.dram_tensor("wq", [DIM, INNER], BF16, kind="ExternalInput")
    wk_d = nc.dram_tensor("wk", [DIM, INNER], BF16, kind="ExternalInput")
    wv_d = nc.dram_tensor("wv", [DIM, INNER], BF16, kind="ExternalInput")
    wp_d = nc.dram_tensor("wp", [INNER, DIM], BF16, kind="ExternalInput")
    ones_d = nc.dram_tensor("ones_col", [128, 1], BF16, kind="ExternalInput")
    if with_bias:
        bq_d = nc.dram_tensor("bq", [INNER, 1], F32, kind="ExternalInput")
        bk_d = nc.dram_tensor("bk", [INNER, 1], F32, kind="ExternalInput")
        bv_d = nc.dram_tensor("bv_row", [1, INNER], BF16, kind="ExternalInput")
        bp_d = nc.dram_tensor("bp6", [DIM, 1], F32, kind="ExternalInput")
    out_d = nc.dram_tensor("out", [n_win, DIM, WQ], F32, kind="ExternalOutput")

    with tile.TileContext(nc) as tc:
        with (
            tc.tile_pool(name="singles", bufs=1) as singles,
            tc.tile_pool(name="xbuf", bufs=2) as xbuf,
            tc.tile_pool(name="lnb", bufs=2) as lnb,
            tc.tile_pool(name="stats", bufs=2) as statp,
            tc.tile_pool(name="xt", bufs=2) as xtp,
            tc.tile_pool(name="qkv", bufs=2) as qkvp,
            tc.tile_pool(name="esb", bufs=2) as esbp,
            tc.tile_pool(name="small", bufs=2) as smallp,
            tc.tile_pool(name="ps_p", bufs=2, space="PSUM") as ps_p,
            tc.tile_pool(name="ps_s", bufs=1, space="PSUM") as ps_s,
            tc.tile_pool(name="ps_a", bufs=1, space="PSUM") as ps_a,
        ):
            # ---- constants / weights ----
            ones_col = singles.tile([128, 1], BF16)
            nc.sync.dma_start(ones_col, ones_d[:, :])
            eps_sb = singles.tile([128, 1], F32)
            nc.vector.memset(eps_sb, EPS)
            zero_sb = singles.tile([128, 1], F32)
            nc.vector.memset(zero_sb, 0.0)
            w_sb = {}
            for nm, d in (("q", wq_d), ("k", wk_d), ("v", wv_d), ("p", wp_d)):
                w_sb[nm] = singles.tile([128, 128], BF16, name=f"w{nm}", tag=f"w{nm}")
                nc.sync.dma_start(w_sb[nm], d[:, :])
            if with_bias:
                bq_sb = singles.tile([INNER, 1], F32)
                nc.sync.dma_start(bq_sb, bq_d[:, :])
                bk_sb = singles.tile([INNER, 1], F32)
                nc.sync.dma_start(bk_sb, bk_d[:, :])
                bv_sb = singles.tile([1, INNER], BF16)
                nc.sync.dma_start(bv_sb, bv_d[:, :])
                bp_sb = singles.tile([DIM, 1], F32)
                nc.sync.dma_start(bp_sb, bp_d[:, :])
                ones_row128 = singles.tile([1, 128], BF16)
                nc.vector.memset(ones_row128, 1.0)
            zall = singles.tile([128, n_win, WQ], F32, name="zall")

            # software-pipelined input loads: issue window w+1's DMA during
            # window w so it never queues behind w's late-stage SP DMAs
            x_tiles = {}
            x_tiles[0] = xbuf.tile([128, 3, TCH, DIM], BF16, tag="x", name="x")
            nc.sync.dma_start(x_tiles[0][:, :, :, :], xin_d[0, :, :, :, :])
            for w in range(n_win):
                x_sb = x_tiles.pop(w)
                if w + 1 < n_win:
                    x_tiles[w + 1] = xbuf.tile([128, 3, TCH, DIM], BF16,
                                               tag="x", name="x")
                    nc.sync.dma_start(x_tiles[w + 1][:, :, :, :],
                                      xin_d[w + 1, :, :, :, :])
                stats6 = statp.tile([128, 9, 6], F32, tag="st6")
                for ti in range(3):
                    for c in range(TCH):
                        nc.vector.bn_stats(
                            out=stats6[:, 3 * ti + c, :], in_=x_sb[:, ti, c, :]
                        )
                mv = statp.tile([128, 9, 2], F32, tag="mv")
                for g in range(9):
                    nc.vector.bn_aggr(out=mv[:, g, :], in_=stats6[:, g, :])
                # rstd = exp(-0.5 * ln(var + EPS))
                lnv = statp.tile([128, 9], F32, tag="lnv")
                nc.scalar.activation(
                    out=lnv, in_=mv[:, :, 1], func=ACT.Ln, bias=eps_sb, scale=1.0
                )
                rstd = statp.tile([128, 9], F32, tag="rstd")
                nc.scalar.activation(
                    out=rstd, in_=lnv, func=ACT.Exp, bias=zero_sb, scale=-0.5
                )
                # LN apply on GPSIMD (SBUF->SBUF)
                xl = lnb.tile([128, 3, TCH, DIM], BF16, tag="xl", name="xl")
                for ti in range(3):
                    for c in range(TCH):
                        g = 3 * ti + c
                        nc.gpsimd.tensor_scalar(
                            out=xl[:, ti, c, :], in0=x_sb[:, ti, c, :],
                            scalar1=mv[:, g, 0:1], scalar2=rstd[:, g:g + 1],
                            op0=ALU.subtract, op1=ALU.mult,
                        )

                # ---- x^T via one SBUF->SBUF xbar DMA transpose ----
                # xt[:, ti, c, :] = xl[:, ti, c, :]^T  -> [128(d), 3, TCH, 128(t)]
                xt_ = xtp.tile([128, 3, TCH, 128], BF16, tag="xt", name="xt")
                nc.sync.dma_start_transpose(
                    xt_[:, :, :, :], xl[:, :, :, :]
                )

                # ---- projections (bf16 weights, fp32 PSUM) ----
                qT = qkvp.tile([128, T], BF16, tag="qT")
                kT = qkvp.tile([128, T], BF16, tag="kT")
                for ti, (nm, dst) in enumerate((("q", qT), ("k", kT))):
                    pps = ps_p.tile([128, T], F32, tag="pp")
                    nc.tensor.matmul(pps, w_sb[nm], xt_[:, ti, :, :])
                    if with_bias:
                        bb = bq_sb if nm == "q" else bk_sb
                        nc.vector.tensor_scalar(
                            out=dst, in0=pps, scalar1=bb, scalar2=None, op0=ALU.add
                        )
                    elif nm == "q":
                        with nc.allow_low_precision(reason="qT to bf16"):
                            nc.scalar.copy(dst, pps)
                    else:
                        nc.vector.tensor_copy(dst, pps)
                v_sb = qkvp.tile([128, TCH, DIM], BF16, tag="v")
                vps = ps_p.tile([128, TCH, DIM], F32, tag="pp")
                for c in range(TCH):
                    nc.tensor.matmul(
                        vps[:, c, :], xt_[:, 2, c, :], w_sb["v"]
                    )
                    if with_bias:
                        nc.tensor.matmul(
                            vps[:, c, :], ones_row128, bv_sb, start=False
                        )
                nc.vector.tensor_copy(v_sb, vps)

                # ---- attention: scores -> exp -> A[q, dh] + den ----
                a_ps = ps_a.tile([128, TCH, HEADS, DH], F32, tag="A")
                dz_ps = ps_a.tile([128, 12 + WQ], F32, tag="dz")
                # head-pair tiles double-buffer so exp(pair p) overlaps
                # scores(pair p+1) on the PE
                for ck in range(TCH):
                    for pr in range(2):
                        s_ps = ps_s.tile([128, 2, 512], F32, tag="S", bufs=2)
                        for hh in range(2):
                            h = 2 * pr + hh
                            nc.tensor.matmul(
                                s_ps[:, hh, 0:T],
                                kT[32 * h:32 * h + 32, 128 * ck:128 * (ck + 1)],
                                qT[32 * h:32 * h + 32, :],
                                tile_position=(32 * h, 0),
                            )
                        e_sb = esbp.tile([128, 2, T], BF16, tag=f"E{ck}{pr}")
                        ncols = DVE_EXP_COLS if ck == TCH - 1 else 0
                        if ncols:
                            e_i = e_sb.bitcast(I16)
                            nc.vector.tensor_scalar(
                                out=e_i[:, :, T - ncols:T],
                                in0=s_ps[:, :, T - ncols:T],
                                scalar1=EXPA * SCALE, scalar2=EXPB,
                                op0=ALU.mult, op1=ALU.add,
                            )
                        nc.scalar.activation(
                            out=e_sb[:, :, 0:T - ncols],
                            in_=s_ps[:, :, 0:T - ncols],
                            func=ACT.Exp, bias=zero_sb, scale=SCALE,
                        )
                        for cq in range(TCH):
                            for hh in range(2):
                                h = 2 * pr + hh
                                # PSUM start=True zeroes the whole 2KB bank
                                # (ZERO_REGION_SIZE): issue it exactly once
                                # per bank per window, others accumulate.
                                first = (ck == 0 and pr == 0 and cq == 0
                                         and hh == 0)
                                sp = (ck == TCH - 1)
                                nc.tensor.matmul(
                                    a_ps[:, cq, h, :],
                                    e_sb[:, hh, 128 * cq:128 * (cq + 1)],
                                    v_sb[:, ck, 32 * h:32 * h + 32],
                                    start=first, stop=sp, skip_group_check=True,
                                )
                                nc.tensor.matmul(
                                    dz_ps[:, 4 * cq + h:4 * cq + h + 1],
                                    e_sb[:, hh, 128 * cq:128 * (cq + 1)],
                                    ones_col,
                                    start=first, stop=sp, skip_group_check=True,
                                )

                # ---- normalize (per-partition scalars, bcast over dh) ----
                r_sb = smallp.tile([128, 12], F32, tag="r")
                nc.vector.reciprocal(r_sb, dz_ps[:, 0:12])
                a_sb = smallp.tile([128, TCH, HEADS, DH], BF16, tag="asb")
                for cq in range(TCH):
                    with nc.allow_low_precision(reason="attn out to bf16"):
                        nc.vector.tensor_tensor(
                            out=a_sb[:, cq, :, :],
                            in0=a_ps[:, cq, :, :],
                            in1=r_sb[:, 4 * cq:4 * cq + 4, None]
                                .broadcast_to([128, HEADS, DH]),
                            op=ALU.mult,
                        )
                # A^T via xbar DMA transpose: at[:, cq, :] = a_sb[:, cq, :]^T
                at_sb = smallp.tile([128, TCH, 128], BF16, tag="atsb")
                nc.sync.dma_start_transpose(
                    at_sb[:, :, :], a_sb[:, :, :, :]
                )

                # ---- output projection with folded n-mean, store z^T ----
                zt_view = dz_ps[:, 12:12 + WQ]
                for n in range(N):
                    cq, t0 = divmod(n * WQ, 128)
                    # dz bank was zeroed by the first den matmul's start
                    nc.tensor.matmul(
                        zt_view, w_sb["p"], at_sb[:, cq, t0:t0 + WQ],
                        start=False, stop=(n == N - 1), skip_group_check=True,
                    )
                if with_bias:
                    nc.scalar.add(zall[:, w, :], zt_view, bp_sb)
                else:
                    nc.scalar.copy(zall[:, w, :], zt_view)
            nc.sync.dma_start(out_d[:, :, :].rearrange("w d t -> d w t"), zall)

    return nc


def split_multi_waits(nc):
    """Walrus encodes at most one sem-wait per instruction on this toolchain;
    move extra waits onto same-engine NoOp carriers placed just before."""
    k = 0
    for f in nc.m.functions:
        for blk in f.blocks:
            new = []
            for inst in blk.instructions:
                si = getattr(inst, "sync_info", None)
                if si and si.on_wait and len(si.on_wait) > 1:
                    waits = list(si.on_wait)
                    for w in waits[:-1]:
                        nop = mybir.InstNoOp(
                            name=f"{inst.name}_wsplit{k}", ins=[], outs=[]
                        )
                        k += 1
                        nop.engine = inst.engine
                        nop.sync_info = mybir.SyncInfo(on_wait=[w], on_update=[])
                        new.append(nop)
                    si.on_wait = [waits[-1]]
                new.append(inst)
            blk.instructions[:] = new
    return nc


def fold_params(inp):
    folded = {}
    for nm in ("q", "k", "v"):
        g = inp[f"ln_{nm}_g"]
        bb = inp[f"ln_{nm}_b"]
        W = inp[f"W{nm}"]
        folded[f"W{nm}"] = np.ascontiguousarray(g[:, None] * W)
        folded[f"b{nm}"] = inp[f"b{nm}"] + bb @ W
    folded["Wp6"] = np.ascontiguousarray(inp["Wp"] / N)
    folded["bp6"] = inp["bp"] / N
    return folded


def make_base_inputs(folded, with_bias):
    identbf, ones_col = host_consts()
    base = {
        "wq": folded["Wq"].astype(NPBF), "wk": folded["Wk"].astype(NPBF),
        "wv": folded["Wv"].astype(NPBF), "wp": folded["Wp6"].astype(NPBF),
        "ones_col": ones_col,
    }
    if with_bias:
        base["bq"] = folded["bq"].reshape(INNER, 1).astype(np.float32)
        base["bk"] = folded["bk"].reshape(INNER, 1).astype(np.float32)
        base["bv_row"] = folded["bv"].reshape(1, INNER).astype(NPBF)
        base["bp6"] = folded["bp6"].reshape(DIM, 1).astype(np.float32)
    return base


def make_xin(inp, core):
    """Pack core `core`'s windows: [n_win, 128, 3, TCH, DIM] bf16."""
    xrows = X // NCORES
    parts = []
    for key in ("q", "k", "v"):
        sh = inp[key][0, :, xrows * core:xrows * (core + 1)]  # [N,2,Y,W1,W2,D]
        sh = sh.reshape(N, WIN_PER_CORE, WQ, DIM).transpose(1, 0, 2, 3)
        sh = sh.reshape(WIN_PER_CORE, TCH, 128, DIM).transpose(0, 2, 1, 3)
        parts.append(sh)  # [n_win, 128, TCH, DIM]
    xin = np.stack(parts, axis=2)  # [n_win, 128, 3, TCH, DIM]
    return np.ascontiguousarray(xin.astype(NPBF))


def kernel(**inputs) -> np.ndarray:
    inp = {k: np.ascontiguousarray(np.asarray(v, dtype=np.float32))
           for k, v in inputs.items()}

    folded = fold_params(inp)
    with_bias = any(
        np.abs(folded[b]).max() > 0 for b in ("bq", "bk", "bv", "bp6")
    )

    nc = build(WIN_PER_CORE, with_bias)
    nc.finalize()
    split_multi_waits(nc)

    base = make_base_inputs(folded, with_bias)
    in_maps = []
    for c in range(NCORES):
        m = dict(base)
        m["xin"] = make_xin(inp, c)
        in_maps.append(m)

    res = run_bass_kernel_spmd(nc, in_maps, core_ids=list(range(NCORES)))
    global LAST_RESULT
    LAST_RESULT = res
    outs = res.results
    xrows = X // NCORES
    full = np.zeros((B, X, Y, W1, W2, DIM), np.float32)
    for c in range(NCORES):
        o = np.asarray(outs[c]["out"])          # [n_win, DIM, WQ]
        o = o.transpose(0, 2, 1)                # [n_win, WQ, DIM]
        o = o.reshape(xrows, Y, W1, W2, DIM)
        full[0, xrows * c:xrows * (c + 1)] = o
    return full


# revision 13
# speedup vs baseline: 1.1315x; 1.1292x over previous
"""CrossWinAttention Trainium2 kernel, v2 (bf16 DMA + [q,dh] attention layout).

Computes, for each of 256 independent (x,y) windows:
  LN -> Q/K/V projections -> 4-head attention over T=384 tokens
  -> output projection -> mean over the N=6 slices.

Sharding: 8 cores x 32 windows. LN affine and linear biases folded into
weights host-side; 1/N mean folded into Wp. Inputs are DMA'd as bf16
(halves HBM traffic; comfortably inside the rel-err budget).

Cost-model-driven engine assignment (PE matmul cost = output free size;
GPSIMD cannot touch PSUM; DMA cannot read PSUM):
  - attention output A in [query, head-dim] layout: out [128q,32] costs
    32 rows/instruction instead of 384 -> A path ~4x cheaper; softmax
    denominators are 1-column matmuls; 1/den is a per-partition scalar.
  - x^T and A^T via one dma_start_transpose each (SBUF->SBUF xbar,
    14ns/tile) instead of PE transposes + PSUM->SBUF bounce copies.
  - LN apply on GPSIMD (SBUF->SBUF), stats on DVE, exp on ACT with an
    optional Schraudolph tail on DVE (exp(x) ~ bitcast(int16(x*a+b))).
  - output stays z^T [dim, wq]; the host transposes on unshard.
"""

import ml_dtypes
import numpy as np

import concourse.bass as bass
import concourse.tile as tile
from concourse import mybir
from concourse.bass_utils import run_bass_kernel_spmd

# Problem shape (hardcoded per spec)
B, N, X, Y, W1, W2 = 1, 6, 16, 16, 8, 8
DIM, HEADS, DH = 128, 4, 32
INNER = HEADS * DH
T = N * W1 * W2          # 384 tokens per window
WQ = W1 * W2             # 64 tokens per n-slice
NCORES = 8
WIN_PER_CORE = (X // NCORES) * Y   # 32
EPS = 1e-5
SCALE = DH ** -0.5
F32 = mybir.dt.float32
BF16 = mybir.dt.bfloat16
I16 = mybir.dt.int16
NPBF = ml_dtypes.bfloat16
ACT = mybir.ActivationFunctionType
ALU = mybir.AluOpType

TCH = T // 128           # 3 token chunks of 128

# Schraudolph exp in bf16: exp(x) ~ bitcast_bf16(int16(x*EXPA + EXPB))
EXPA = 128.0 / np.log(2.0)
EXPB = 16249.5
# columns of the last key-chunk's scores that take the DVE exp path
DVE_EXP_COLS = 64

LAST_RESULT = None       # BassKernelResults of the most recent kernel() call


def host_consts():
    identbf = np.eye(128, dtype=np.float32).astype(NPBF)
    ones_col = np.ones((128, 1), np.float32).astype(NPBF)
    return identbf, ones_col


def build(n_win: int, with_bias: bool):
    """Build the per-core Bass program. Inputs are per-core shards.

    xin: [n_win, 128, 3, TCH, DIM] bf16 (q,k,v tensor-major, token chunks)
    out: [n_win, DIM, WQ] fp32  (z^T per window; host transposes)
    """
    nc = bass.Bass()

    xin_d = nc.dram_tensor("xin", [n_win, 128, 3, TCH, DIM], BF16,
                           kind="ExternalInput")
    wq_d = nc.dram_tensor("wq", [DIM, INNER], BF16, kind="ExternalInput")
    wk_d = nc.dram_tensor("wk", [DIM, INNER], BF16, kind="ExternalInput")
    wv_d = nc.dram_tensor("wv", [DIM, INNER], BF16, kind="ExternalInput")
    wp_d = nc.dram_tensor("wp", [INNER, DIM], BF16, kind="ExternalInput")
    ones_d = nc.dram_tensor("ones_col", [128, 1], BF16, kind="ExternalInput")
    if with_bias:
        bq_d = nc.dram_tensor("bq", [INNER, 1], F32, kind="ExternalInput")
        bk_d = nc.dram_tensor("bk", [INNER, 1], F32, kind="ExternalInput")
        bv_d = nc.dram_tensor("bv_row", [1, INNER], BF16, kind="ExternalInput")
        bp_d = nc.dram_tensor("bp6", [DIM, 1], F32, kind="ExternalInput")
    out_d = nc.dram_tensor("out", [n_win, DIM, WQ], F32, kind="ExternalOutput")

    with tile.TileContext(nc) as tc:
        with (
            tc.tile_pool(name="singles", bufs=1) as singles,
            tc.tile_pool(name="xbuf", bufs=2) as xbuf,
            tc.tile_pool(name="lnb", bufs=2) as lnb,
            tc.tile_pool(name="stats", bufs=2) as statp,
            tc.tile_pool(name="xt", bufs=2) as xtp,
            tc.tile_pool(name="qkv", bufs=2) as qkvp,
            tc.tile_pool(name="esb", bufs=2) as esbp,
            tc.tile_pool(name="small", bufs=2) as smallp,
            tc.tile_pool(name="ps_p", bufs=2, space="PSUM") as ps_p,
            tc.tile_pool(name="ps_s", bufs=1, space="PSUM") as ps_s,
            tc.tile_pool(name="ps_a", bufs=1, space="PSUM") as ps_a,
        ):
            # ---- constants / weights ----
            ones_col = singles.tile([128, 1], BF16)
            nc.sync.dma_start(ones_col, ones_d[:, :])
            eps_sb = singles.tile([128, 1], F32)
            nc.vector.memset(eps_sb, EPS)
            zero_sb = singles.tile([128, 1], F32)
            nc.vector.memset(zero_sb, 0.0)
            w_sb = {}
            for nm, d in (("q", wq_d), ("k", wk_d), ("v", wv_d), ("p", wp_d)):
                w_sb[nm] = singles.tile([128, 128], BF16, name=f"w{nm}", tag=f"w{nm}")
                nc.sync.dma_start(w_sb[nm], d[:, :])
            if with_bias:
                bq_sb = singles.tile([INNER, 1], F32)
                nc.sync.dma_start(bq_sb, bq_d[:, :])
                bk_sb = singles.tile([INNER, 1], F32)
                nc.sync.dma_start(bk_sb, bk_d[:, :])
                bv_sb = singles.tile([1, INNER], BF16)
                nc.sync.dma_start(bv_sb, bv_d[:, :])
                bp_sb = singles.tile([DIM, 1], F32)
                nc.sync.dma_start(bp_sb, bp_d[:, :])
                ones_row128 = singles.tile([1, 128], BF16)
                nc.vector.memset(ones_row128, 1.0)
            zall = singles.tile([128, n_win, WQ], F32, name="zall")

            def emit_backend(state):
                pw, pa_sb, pdz = state
                # A^T via xbar DMA transpose: at[:, cq, :] = a_sb[:, cq, :]^T
                at_sb = smallp.tile([128, TCH, 128], BF16, tag="atsb",
                                    name="at_sb")
                nc.sync.dma_start_transpose(
                    at_sb[:, :, :], pa_sb[:, :, :, :]
                )
                # output projection with folded n-mean -> z^T staging
                zt_view = pdz[:, 12:12 + WQ]
                for n in range(N):
                    cq, t0 = divmod(n * WQ, 128)
                    # dz bank was zeroed by the first den matmul's start
                    nc.tensor.matmul(
                        zt_view, w_sb["p"], at_sb[:, cq, t0:t0 + WQ],
                        start=False, stop=(n == N - 1), skip_group_check=True,
                    )
                if with_bias:
                    nc.vector.tensor_scalar(
                        out=zall[:, pw, :], in0=zt_view, scalar1=bp_sb,
                        scalar2=None, op0=ALU.add,
                    )
                else:
                    nc.vector.tensor_copy(zall[:, pw, :], zt_view)

            # software pipelining: window w's body emits w's front-end
            # (stats/LN/x^T), then window w-1's deferred back-end (A^T, z,
            # out-staging), then w's attention. This keeps the SP DMA FIFO
            # in data-readiness order (xlT(w) ahead of atT(w-1), in(w+1)).
            x_tiles = {}
            x_tiles[0] = xbuf.tile([128, 3, TCH, DIM], BF16, tag="x", name="x")
            nc.sync.dma_start(x_tiles[0][:, :, :, :], xin_d[0, :, :, :, :])
            prev = None
            for w in range(n_win):
                x_sb = x_tiles.pop(w)
                stats6 = statp.tile([128, 9, 6], F32, tag="st6")
                for ti in range(3):
                    for c in range(TCH):
                        nc.vector.bn_stats(
                            out=stats6[:, 3 * ti + c, :], in_=x_sb[:, ti, c, :]
                        )
                mv = statp.tile([128, 9, 2], F32, tag="mv")
                for g in range(9):
                    nc.vector.bn_aggr(out=mv[:, g, :], in_=stats6[:, g, :])
                # rstd = exp(-0.5 * ln(var + EPS))
                lnv = statp.tile([128, 9], F32, tag="lnv")
                nc.scalar.activation(
                    out=lnv, in_=mv[:, :, 1], func=ACT.Ln, bias=eps_sb, scale=1.0
                )
                rstd = statp.tile([128, 9], F32, tag="rstd")
                nc.scalar.activation(
                    out=rstd, in_=lnv, func=ACT.Exp, bias=zero_sb, scale=-0.5
                )
                # LN apply on GPSIMD (SBUF->SBUF)
                xl = lnb.tile([128, 3, TCH, DIM], BF16, tag="xl", name="xl")
                for ti in range(3):
                    for c in range(TCH):
                        g = 3 * ti + c
                        nc.gpsimd.tensor_scalar(
                            out=xl[:, ti, c, :], in0=x_sb[:, ti, c, :],
                            scalar1=mv[:, g, 0:1], scalar2=rstd[:, g:g + 1],
                            op0=ALU.subtract, op1=ALU.mult,
                        )

                # ---- x^T via one SBUF->SBUF xbar DMA transpose ----
                # xt[:, ti, c, :] = xl[:, ti, c, :]^T  -> [128(d), 3, TCH, 128(t)]
                xt_ = xtp.tile([128, 3, TCH, 128], BF16, tag="xt", name="xt")
                nc.sync.dma_start_transpose(
                    xt_[:, :, :, :], xl[:, :, :, :]
                )

                # ---- deferred back-end of window w-1 ----
                if prev is not None:
                    emit_backend(prev)
                    prev = None
                if w + 1 < n_win:
                    x_tiles[w + 1] = xbuf.tile([128, 3, TCH, DIM], BF16,
                                               tag="x", name="x")
                    nc.sync.dma_start(x_tiles[w + 1][:, :, :, :],
                                      xin_d[w + 1, :, :, :, :])

                # ---- projections (bf16 weights, fp32 PSUM) ----
                qT = qkvp.tile([128, T], BF16, tag="qT")
                kT = qkvp.tile([128, T], BF16, tag="kT")
                for ti, (nm, dst) in enumerate((("q", qT), ("k", kT))):
                    pps = ps_p.tile([128, T], F32, tag="pp")
                    nc.tensor.matmul(pps, w_sb[nm], xt_[:, ti, :, :])
                    if with_bias:
                        bb = bq_sb if nm == "q" else bk_sb
                        nc.vector.tensor_scalar(
                            out=dst, in0=pps, scalar1=bb, scalar2=None, op0=ALU.add
                        )
                    else:
                        nc.vector.tensor_copy(dst, pps)
                v_sb = qkvp.tile([128, TCH, DIM], BF16, tag="v")
                vps = ps_p.tile([128, TCH, DIM], F32, tag="pp")
                for c in range(TCH):
                    nc.tensor.matmul(
                        vps[:, c, :], xt_[:, 2, c, :], w_sb["v"]
                    )
                    if with_bias:
                        nc.tensor.matmul(
                            vps[:, c, :], ones_row128, bv_sb, start=False
                        )
                nc.vector.tensor_copy(v_sb, vps)

                # ---- attention: scores -> exp -> A[q, dh] + den ----
                a_ps = ps_a.tile([128, TCH, HEADS, DH], F32, tag="A")
                dz_ps = ps_a.tile([128, 12 + WQ], F32, tag="dz")
                # head-pair tiles double-buffer so exp(pair p) overlaps
                # scores(pair p+1) on the PE
                for ck in range(TCH):
                    for pr in range(2):
                        s_ps = ps_s.tile([128, 2, 512], F32, tag="S", bufs=2)
                        for hh in range(2):
                            h = 2 * pr + hh
                            nc.tensor.matmul(
                                s_ps[:, hh, 0:T],
                                kT[32 * h:32 * h + 32, 128 * ck:128 * (ck + 1)],
                                qT[32 * h:32 * h + 32, :],
                                tile_position=(32 * h, 0),
                            )
                        e_sb = esbp.tile([128, 2, T], BF16, tag=f"E{ck}{pr}")
                        ncols = DVE_EXP_COLS if ck == TCH - 1 else 0
                        if ncols:
                            e_i = e_sb.bitcast(I16)
                            nc.vector.tensor_scalar(
                                out=e_i[:, :, T - ncols:T],
                                in0=s_ps[:, :, T - ncols:T],
                                scalar1=EXPA * SCALE, scalar2=EXPB,
                                op0=ALU.mult, op1=ALU.add,
                            )
                        nc.scalar.activation(
                            out=e_sb[:, :, 0:T - ncols],
                            in_=s_ps[:, :, 0:T - ncols],
                            func=ACT.Exp, bias=zero_sb, scale=SCALE,
                        )
                        for cq in range(TCH):
                            for hh in range(2):
                                h = 2 * pr + hh
                                # PSUM start=True zeroes the whole 2KB bank
                                # (ZERO_REGION_SIZE): issue it exactly once
                                # per bank per window, others accumulate.
                                first = (ck == 0 and pr == 0 and cq == 0
                                         and hh == 0)
                                sp = (ck == TCH - 1)
                                nc.tensor.matmul(
                                    a_ps[:, cq, h, :],
                                    e_sb[:, hh, 128 * cq:128 * (cq + 1)],
                                    v_sb[:, ck, 32 * h:32 * h + 32],
                                    start=first, stop=sp, skip_group_check=True,
                                )
                                nc.tensor.matmul(
                                    dz_ps[:, 4 * cq + h:4 * cq + h + 1],
                                    e_sb[:, hh, 128 * cq:128 * (cq + 1)],
                                    ones_col,
                                    start=first, stop=sp, skip_group_check=True,
                                )

                # ---- normalize (per-partition scalars, bcast over dh) ----
                r_sb = smallp.tile([128, 12], F32, tag="r")
                nc.vector.reciprocal(r_sb, dz_ps[:, 0:12])
                a_sb = smallp.tile([128, TCH, HEADS, DH], BF16, tag="asb")
                for cq in range(TCH):
                    with nc.allow_low_precision(reason="attn out to bf16"):
                        nc.vector.tensor_tensor(
                            out=a_sb[:, cq, :, :],
                            in0=a_ps[:, cq, :, :],
                            in1=r_sb[:, 4 * cq:4 * cq + 4, None]
                                .broadcast_to([128, HEADS, DH]),
                            op=ALU.mult,
                        )
                prev = (w, a_sb, dz_ps)
            if prev is not None:
                emit_backend(prev)
            nc.sync.dma_start(out_d[:, :, :].rearrange("w d t -> d w t"), zall)

    return nc


def split_multi_waits(nc):
    """Walrus encodes at most one sem-wait per instruction on this toolchain;
    move extra waits onto same-engine NoOp carriers placed just before."""
    k = 0
    for f in nc.m.functions:
        for blk in f.blocks:
            new = []
            for inst in blk.instructions:
                si = getattr(inst, "sync_info", None)
                if si and si.on_wait and len(si.on_wait) > 1:
                    waits = list(si.on_wait)
                    for w in waits[:-1]:
                        nop = mybir.InstNoOp(
                            name=f"{inst.name}_wsplit{k}", ins=[], outs=[]
                        )
                        k += 1
                        nop.engine = inst.engine
                        nop.sync_info = mybir.SyncInfo(on_wait=[w], on_update=[])
                        new.append(nop)
                    si.on_wait = [waits[-1]]
                new.append(inst)
            blk.instructions[:] = new
    return nc


def fold_params(inp):
    folded = {}
    for nm in ("q", "k", "v"):
        g = inp[f"ln_{nm}_g"]
        bb = inp[f"ln_{nm}_b"]
        W = inp[f"W{nm}"]
        folded[f"W{nm}"] = np.ascontiguousarray(g[:, None] * W)
        folded[f"b{nm}"] = inp[f"b{nm}"] + bb @ W
    folded["Wp6"] = np.ascontiguousarray(inp["Wp"] / N)
    folded["bp6"] = inp["bp"] / N
    return folded


def make_base_inputs(folded, with_bias):
    identbf, ones_col = host_consts()
    base = {
        "wq": folded["Wq"].astype(NPBF), "wk": folded["Wk"].astype(NPBF),
        "wv": folded["Wv"].astype(NPBF), "wp": folded["Wp6"].astype(NPBF),
        "ones_col": ones_col,
    }
    if with_bias:
        base["bq"] = folded["bq"].reshape(INNER, 1).astype(np.float32)
        base["bk"] = folded["bk"].reshape(INNER, 1).astype(np.float32)
        base["bv_row"] = folded["bv"].reshape(1, INNER).astype(NPBF)
        base["bp6"] = folded["bp6"].reshape(DIM, 1).astype(np.float32)
    return base


def make_xin(inp, core):
    """Pack core `core`'s windows: [n_win, 128, 3, TCH, DIM] bf16."""
    xrows = X // NCORES
    parts = []
    for key in ("q", "k", "v"):
        sh = inp[key][0, :, xrows * core:xrows * (core + 1)]  # [N,2,Y,W1,W2,D]
        sh = sh.reshape(N, WIN_PER_CORE, WQ, DIM).transpose(1, 0, 2, 3)
        sh = sh.reshape(WIN_PER_CORE, TCH, 128, DIM).transpose(0, 2, 1, 3)
        parts.append(sh)  # [n_win, 128, TCH, DIM]
    xin = np.stack(parts, axis=2)  # [n_win, 128, 3, TCH, DIM]
    return np.ascontiguousarray(xin.astype(NPBF))


def kernel(**inputs) -> np.ndarray:
    inp = {k: np.ascontiguousarray(np.asarray(v, dtype=np.float32))
           for k, v in inputs.items()}

    folded = fold_params(inp)
    with_bias = any(
        np.abs(folded[b]).max() > 0 for b in ("bq", "bk", "bv", "bp6")
    )

    nc = build(WIN_PER_CORE, with_bias)
    nc.finalize()
    split_multi_waits(nc)

    base = make_base_inputs(folded, with_bias)
    in_maps = []
    for c in range(NCORES):
        m = dict(base)
        m["xin"] = make_xin(inp, c)
        in_maps.append(m)

    res = run_bass_kernel_spmd(nc, in_maps, core_ids=list(range(NCORES)))
    global LAST_RESULT
    LAST_RESULT = res
    outs = res.results
    xrows = X // NCORES
    full = np.zeros((B, X, Y, W1, W2, DIM), np.float32)
    for c in range(NCORES):
        o = np.asarray(outs[c]["out"])          # [n_win, DIM, WQ]
        o = o.transpose(0, 2, 1)                # [n_win, WQ, DIM]
        o = o.reshape(xrows, Y, W1, W2, DIM)
        full[0, xrows * c:xrows * (c + 1)] = o
    return full
